# revision 2
# baseline (speedup 1.0000x reference)
"""Enformer relative-position attention block on 8 Trainium2 NeuronCores.

Sharding: core c handles batch b = c//4 and head pair hp = c%4 (heads 2hp,
2hp+1).  Wq/Wk/Wv/W_rel_k sliced column-wise per head pair, Wo row-wise; x
sharded by batch.  Each core computes a partial (n, dim) output in bf16; the
host sums the 4 partials per batch (fp32) and adds bo.

v2 design (vs fp32r baseline):
  - all matmul operands bf16 (logits accumulate fp32 in PSUM)
  - x transposed DRAM->SBUF by the DMA XBAR (no PE transposes at all)
  - softmax factored: p = exp(content) * exp(rel): the relative shift is a
    diagonal SBUF->SBUF DMA of exp(rel window); the product runs on DVE
  - p transposed per q-chunk by one tiled XBAR DMA; attention output
    accumulated as o[q, dv] with a ones-column in V producing the softmax
    row-sums for free; normalization folded into the PSUM->SBUF scale-copy
  - output projection interleaved per chunk right after its oT transpose
"""

import math

import numpy as np
import ml_dtypes

import concourse.bass as bass
import concourse.mybir as mybir
from concourse import bacc
from concourse.tile import TileContext
from concourse.bass_utils import run_bass_kernel_spmd

F32 = mybir.dt.float32
BF16 = mybir.dt.bfloat16
BF = ml_dtypes.bfloat16

HEADS, DIM, DK, DV, NRPF = 8, 1536, 64, 192, 192
N = 1536
NCH = N // 128           # 12 query chunks
DIMCH = DIM // 128       # 12 contraction chunks
TWO_N1 = 2 * N - 1       # 3071
POSW = 3072              # padded rel-position width
WIN = 1664               # padded per-chunk window width
WSTR = WIN - 1           # diagonal read row stride
SCALE = DK ** -0.5
VW = 193                 # per-head value width incl. ones column
EXP = mybir.ActivationFunctionType.Exp
IDENT = mybir.ActivationFunctionType.Identity


def _get_positional_embed_np(n, feature_size):
    """numpy mirror of the reference's jax positional embedding (float64)."""
    from scipy.special import gammaln, xlogy

    nb = feature_size // 6
    dist = np.arange(-n + 1, n, dtype=np.float64)
    ad = np.abs(dist)[:, None]

    max_range = math.log(n) / math.log(2.0)
    half_life = 2.0 ** np.linspace(3.0, max_range, nb)
    f_exp = np.exp(-math.log(2.0) / half_life[None, :] * ad)

    center_widths = 2.0 ** np.arange(1, nb + 1, dtype=np.float64) - 1.0
    f_cm = (center_widths[None, :] > ad).astype(np.float64)

    stddev = n / (2.0 * nb)
    start_mean = n / nb
    mean = np.linspace(start_mean, float(n), nb)[None, :]
    concentration = (mean / stddev) ** 2
    rate = mean / (stddev**2)
    log_unnorm = xlogy(concentration - 1.0, ad) - rate * ad
    log_norm = gammaln(concentration) - concentration * np.log(rate)
    probs = np.exp(log_unnorm - log_norm) + 1e-8
    f_g = probs / np.max(probs)

    emb = np.concatenate([f_exp, f_cm, f_g], axis=-1)
    return np.concatenate([emb, np.sign(dist)[:, None] * emb], axis=-1)


def build_nc(reps=1, debug=False):
    if debug:
        nc = bacc.Bacc("TRN2", target_bir_lowering=False, debug=True)
    else:
        nc = bacc.Bacc(None)

    x_d = nc.declare_dram_parameter("x_b", [N, DIM], BF16, isOutput=False)
    wq_d = nc.declare_dram_parameter("wq_s", [DIM, 128], BF16, isOutput=False)
    wk_d = nc.declare_dram_parameter("wk_s", [DIM, 128], BF16, isOutput=False)
    wv_d = nc.declare_dram_parameter("wv_s", [DIM, 384], BF16, isOutput=False)
    wrk_d = nc.declare_dram_parameter("wrk_s", [NRPF, 128], BF16, isOutput=False)
    post_d = nc.declare_dram_parameter("posT", [NRPF, POSW], BF16, isOutput=False)
    bc_d = nc.declare_dram_parameter("bc_s", [128], F32, isOutput=False)
    bp_d = nc.declare_dram_parameter("bp_s", [128], F32, isOutput=False)
    wo_d = nc.declare_dram_parameter("wo_s", [384, DIM], BF16, isOutput=False)
    out_d = nc.declare_dram_parameter("out_p", [N, DIM], BF16, isOutput=True)

    with TileContext(nc) as tc:
      for _rep in range(reps):
        with tc.tile_pool(name="persist", bufs=1) as persist:
            bc_t = persist.tile([128, 1], F32, name="bc", tag="bc")
            bp_t = persist.tile([128, 1], F32, name="bp", tag="bp")
            nc.sync.dma_start(out=bc_t[:], in_=bc_d.rearrange("(p o) -> p o", o=1))
            nc.sync.dma_start(out=bp_t[:], in_=bp_d.rearrange("(p o) -> p o", o=1))

            qc_all = persist.tile([128, N], BF16, name="qc", tag="qc")
            qp_all = persist.tile([128, N], BF16, name="qp", tag="qp")
            k_all = persist.tile([128, N], BF16, name="k", tag="k")
            relk = persist.tile([128, POSW], BF16, name="relk", tag="relk")
            v_sb = [persist.tile([128, 2 * VW], BF16, name=f"v{r}", tag=f"v{r}")
                    for r in range(NCH)]
            oT_all = persist.tile([128, 3 * N], BF16, name="oT", tag="oT")
            wo_t = [persist.tile([128, N], BF16, name=f"wo{kc}", tag=f"wo{kc}")
                    for kc in range(3)]

            # ---------------- xT + q/k/v + rel_k projections ----------------
            with tc.tile_pool(name="pos", bufs=1) as pos_pool, \
                 tc.tile_pool(name="wqkv", bufs=1) as wqkv, \
                 tc.tile_pool(name="xts", bufs=1) as xts_pool, \
                 tc.tile_pool(name="ps_rk", bufs=2, space="PSUM") as ps_rk, \
                 tc.tile_pool(name="ps_qk", bufs=2, space="PSUM") as ps_qk, \
                 tc.tile_pool(name="ps_v", bufs=2, space="PSUM") as ps_v:
                wq_t = wqkv.tile([128, DIMCH * 128], BF16, name="wq", tag="wq")
                wk_t = wqkv.tile([128, DIMCH * 128], BF16, name="wk", tag="wk")
                wv_t = wqkv.tile([128, DIMCH * 384], BF16, name="wv", tag="wv")

                def gather_w(wt, wd, c, rr0, nrr, eng):
                    # [dim, c] DRAM slice rows rr0*128..(rr0+nrr)*128 -> tile cols
                    eng.dma_start(
                        out=bass.AP(tensor=wt[:].tensor, offset=rr0 * c,
                                    ap=[[DIMCH * c, 128], [c, nrr], [1, c]]),
                        in_=bass.AP(tensor=wd, offset=rr0 * 128 * c,
                                    ap=[[c, 128], [128 * c, nrr], [1, c]]))

                # wq first (gates first psq), split across both HWDGE rings
                gather_w(wq_t, wq_d, 128, 0, 6, nc.sync)
                gather_w(wq_t, wq_d, 128, 6, 6, nc.scalar)

                xT = [xts_pool.tile([128, N], BF16, name=f"xT{rr}", tag=f"xT{rr}")
                      for rr in range(DIMCH)]
                for rr in range(DIMCH):
                    # all XBAR transposes stay on the sync ring: concurrent
                    # XBAR streams from both HWDGE rings corrupt data
                    nc.sync.dma_start(out=xT[rr][:],
                                      in_=x_d[:, 128 * rr:128 * rr + 128],
                                      transpose=True)
                gather_w(wk_t, wk_d, 128, 0, 6, nc.sync)
                gather_w(wk_t, wk_d, 128, 6, 6, nc.scalar)
                gather_w(wv_t, wv_d, 384, 0, 6, nc.sync)
                gather_w(wv_t, wv_d, 384, 6, 6, nc.scalar)

                # rel-pos inputs (needed ~20us in) and Wo (needed ~60us in)
                pos_a = pos_pool.tile([128, POSW], BF16, name="posA", tag="posA")
                pos_b = pos_pool.tile([64, POSW], BF16, name="posB", tag="posB")
                wrk_a = pos_pool.tile([128, 128], BF16, name="wrkA", tag="wrkA")
                wrk_b = pos_pool.tile([64, 128], BF16, name="wrkB", tag="wrkB")
                nc.sync.dma_start(out=wrk_a[:], in_=wrk_d[0:128, :])
                nc.sync.dma_start(out=wrk_b[:], in_=wrk_d[128:NRPF, :])
                nc.sync.dma_start(out=pos_a[:], in_=post_d[0:128, :])
                nc.sync.dma_start(out=pos_b[:], in_=post_d[128:NRPF, :])
                for kc in range(3):
                    nc.sync.dma_start(out=wo_t[kc][:],
                                      in_=wo_d[128 * kc:128 * kc + 128, :])

                for s in range(3):
                    if s == 1:
                        # rel_k projection (emitted once q/k/v super 0 is going)
                        for cb in range(6):
                            c0 = 512 * cb
                            ps = ps_rk.tile([128, 512], F32, name="psrk", tag="psrk")
                            nc.tensor.matmul(ps[:], wrk_a[:], pos_a[:, c0:c0 + 512],
                                             start=True, stop=False)
                            nc.tensor.matmul(ps[:], wrk_b[:], pos_b[:, c0:c0 + 512],
                                             start=False, stop=True)
                            nc.vector.tensor_copy(relk[:, c0:c0 + 512], ps[:])
                    cs = slice(512 * s, 512 * s + 512)
                    psq = ps_qk.tile([128, 512], F32, name="psq", tag="psq")
                    psk = ps_qk.tile([128, 512], F32, name="psk", tag="psk")
                    for rr in range(DIMCH):
                        nc.tensor.matmul(psq[:], wq_t[:, 128 * rr:128 * rr + 128],
                                         xT[rr][:, cs],
                                         start=(rr == 0), stop=(rr == DIMCH - 1))
                    nc.scalar.activation(qc_all[:, cs], psq[:], IDENT,
                                         bias=bc_t[:], scale=SCALE)
                    nc.scalar.activation(qp_all[:, cs], psq[:], IDENT,
                                         bias=bp_t[:], scale=SCALE)
                    for rr in range(DIMCH):
                        nc.tensor.matmul(psk[:], wk_t[:, 128 * rr:128 * rr + 128],
                                         xT[rr][:, cs],
                                         start=(rr == 0), stop=(rr == DIMCH - 1))
                    nc.vector.tensor_copy(k_all[:, cs], psk[:])
                    for g in range(4):
                        ci = 4 * s + g
                        psv = ps_v.tile([128, 384], F32, name="psv", tag="psv")
                        for rr in range(DIMCH):
                            nc.tensor.matmul(
                                psv[:],
                                xT[rr][:, 128 * ci:128 * ci + 128],
                                wv_t[:, 384 * rr:384 * rr + 384],
                                start=(rr == 0), stop=(rr == DIMCH - 1))
                        # v columns split per head with a ones column after each
                        nc.vector.tensor_copy(
                            bass.AP(tensor=v_sb[ci][:].tensor, offset=0,
                                    ap=[[2 * VW, 128], [VW, 2], [1, 192]]),
                            bass.AP(tensor=psv[:].tensor, offset=0,
                                    ap=[[384, 128], [192, 2], [1, 192]]))
                        nc.vector.memset(
                            bass.AP(tensor=v_sb[ci][:].tensor, offset=192,
                                    ap=[[2 * VW, 128], [VW, 2], [1, 1]]), 1.0)

            # ------------- attention + interleaved output projection -------------
            # Software-pipelined: unit u = (chunk, head). PE emission order per
            # step: [proj(chunk done 1 step ago)], wwin(u), content(u),
            # AV(u-LAG) — so the exp->diag->mult->XBAR chain of unit u has
            # ~2 units of PE work to hide behind.
            import os as _os
            LAG = int(_os.environ.get("K_LAG", "3"))
            PROJD = int(_os.environ.get("K_PROJD", "3"))
            NU = 2 * NCH
            with tc.tile_pool(name="expw", bufs=3) as ew_pool, \
                 tc.tile_pool(name="rsh", bufs=3) as rsh_pool, \
                 tc.tile_pool(name="ptmp", bufs=3) as pt2_pool, \
                 tc.tile_pool(name="pbuf", bufs=3) as p_pool, \
                 tc.tile_pool(name="ptT", bufs=LAG + 2) as pt_pool, \
                 tc.tile_pool(name="osb", bufs=3) as o_pool, \
                 tc.tile_pool(name="outs", bufs=2) as out_pool, \
                 tc.tile_pool(name="small", bufs=6) as small, \
                 tc.tile_pool(name="ps_w", bufs=1, space="PSUM") as ps_w, \
                 tc.tile_pool(name="ps_cf", bufs=3, space="PSUM") as ps_cf, \
                 tc.tile_pool(name="ps_o", bufs=1, space="PSUM") as ps_o:
                ptT_u = {}
                osb_c = {}
                proj_pending = []
                oT_pending = []

                def emit_oT(ci, osb):
                    nc.sync.dma_start(
                        out=bass.AP(tensor=oT_all[:].tensor, offset=128 * ci,
                                    ap=[[3 * N, 128], [N, 3], [1, 128]]),
                        in_=osb[:], transpose=True)
                    proj_pending.append((ci, cur_t[0]))

                def emit_front(u):
                    ci, h = divmod(u, 2)
                    i0 = 128 * ci
                    w0 = (N - 1) - i0 - 127
                    hs = slice(64 * h, 64 * h + 64)
                    # content logits + exp first (frees pc slots early)
                    p_tmp = pt2_pool.tile([128, N], BF16, name="pt2", tag="pt2")
                    for jb in range(3):
                        j0 = 512 * jb
                        pc = ps_cf.tile([128, 512], F32, name="pc", tag="pcf")
                        nc.tensor.matmul(pc[:], qc_all[hs, i0:i0 + 128],
                                         k_all[hs, j0:j0 + 512],
                                         start=True, stop=True)
                        nc.scalar.activation(p_tmp[:, j0:j0 + 512], pc[:], EXP)
                    # rel window logits: 4 matmuls into 512-strided quarters
                    pw = ps_w.tile([128, 2048], F32, name="pw", tag="pw")
                    for kq in range(4):
                        nc.tensor.matmul(
                            pw[:, 512 * kq:512 * kq + 416],
                            qp_all[hs, i0:i0 + 128],
                            relk[hs, w0 + 416 * kq:w0 + 416 * kq + 416],
                            start=True, stop=True)
                    expw = ew_pool.tile([128, WIN], BF16, name="expw", tag="expw")
                    nc.scalar.activation(
                        bass.AP(tensor=expw[:].tensor, offset=0,
                                ap=[[WIN, 128], [416, 4], [1, 416]]),
                        bass.AP(tensor=pw[:].tensor, offset=0,
                                ap=[[2048, 128], [512, 4], [1, 416]]),
                        EXP)
                    # relative shift: diagonal read of exp(rel window)
                    rsh = rsh_pool.tile([128, N], BF16, name="rsh", tag="rsh")
                    nc.sync.dma_start(
                        out=rsh[:],
                        in_=bass.AP(tensor=expw[:].tensor, offset=127,
                                    ap=[[WSTR, 128], [1, N]]))
                    # p = exp(content) * exp(rel_shifted)  (all-bf16 DVE)
                    p_sb = p_pool.tile([128, N], BF16, name="p", tag="p")
                    nc.vector.tensor_mul(p_sb[:], p_tmp[:], rsh[:])
                    # transpose p via XBAR into 12 [128,128] k-major blocks
                    ptT = pt_pool.tile([128, N], BF16, name="ptT", tag="ptT")
                    nc.sync.dma_start(
                        out=bass.AP(tensor=ptT[:].tensor, offset=0,
                                    ap=[[N, 128], [128, NCH], [1, 128]]),
                        in_=p_sb[:], transpose=True)
                    ptT_u[u] = ptT

                def emit_av(u):
                    ci, h = divmod(u, 2)
                    if h == 0:
                        osb_c[ci] = o_pool.tile([128, 384], BF16,
                                                name="osb", tag="osb")
                    ptT = ptT_u.pop(u)
                    po = ps_o.tile([128, VW], F32, name="po", tag="po")
                    for jb in range(NCH):
                        nc.tensor.matmul(po[:],
                                         ptT[:, 128 * jb:128 * jb + 128],
                                         v_sb[jb][:, VW * h:VW * h + VW],
                                         start=(jb == 0), stop=(jb == NCH - 1))
                    rinv = small.tile([128, 1], F32, name="rinv", tag="rinv")
                    nc.vector.reciprocal(rinv[:], po[:, 192:193])
                    nc.vector.tensor_scalar_mul(osb_c[ci][:, 192 * h:192 * h + 192],
                                                po[:, 0:192], rinv[:])
                    if h == 1:
                        oT_pending.append((ci, osb_c.pop(ci)))

                def emit_proj(ci):
                    i0 = 128 * ci
                    out_sb = out_pool.tile([128, N], BF16, name="outs", tag="outs")
                    for ob in range(3):
                        c0 = 512 * ob
                        pf = ps_cf.tile([128, 512], F32, name="pf", tag="pcf")
                        for kc in range(3):
                            nc.tensor.matmul(pf[:],
                                             oT_all[:, N * kc + i0:N * kc + i0 + 128],
                                             wo_t[kc][:, c0:c0 + 512],
                                             start=(kc == 0), stop=(kc == 2))
                        nc.vector.tensor_copy(out_sb[:, c0:c0 + 512], pf[:])
                    nc.gpsimd.dma_start(out=out_d[i0:i0 + 128, :], in_=out_sb[:])

                cur_t = [0]
                for t in range(NU + LAG + 1):
                    cur_t[0] = t
                    while oT_pending:
                        ci_, osb_ = oT_pending.pop(0)
                        emit_oT(ci_, osb_)
                    if t < NU:
                        emit_front(t)
                    if proj_pending and (t - proj_pending[0][1] >= PROJD or t >= NU):
                        emit_proj(proj_pending.pop(0)[0])
                    if LAG <= t < NU + LAG:
                        emit_av(t - LAG)
                while proj_pending:
                    emit_proj(proj_pending.pop(0)[0])

    nc.compile()
    return nc


_NC_CACHE = None


def _get_nc():
    global _NC_CACHE
    if _NC_CACHE is None:
        _NC_CACHE = build_nc()
    return _NC_CACHE


_POST_CACHE = None


def _get_posT():
    global _POST_CACHE
    if _POST_CACHE is None:
        p = _get_positional_embed_np(N, NRPF).T.astype(np.float32)
        _POST_CACHE = np.zeros((NRPF, POSW), BF)
        _POST_CACHE[:, :TWO_N1] = p.astype(BF)
    return _POST_CACHE


def make_in_maps(inputs):
    x = np.asarray(inputs["x"], np.float32)
    Wq = np.asarray(inputs["Wq"], np.float32)
    Wk = np.asarray(inputs["Wk"], np.float32)
    Wv = np.asarray(inputs["Wv"], np.float32)
    W_rel_k = np.asarray(inputs["W_rel_k"], np.float32)
    bc = np.asarray(inputs["rel_content_bias"], np.float32)[0, :, 0, :]  # (H, DK)
    bp = np.asarray(inputs["rel_pos_bias"], np.float32)[0, :, 0, :]
    Wo = np.asarray(inputs["Wo"], np.float32)
    posT = _get_posT()
    xb = [np.ascontiguousarray(x[b]).astype(BF) for b in range(2)]
    in_maps = []
    for core in range(8):
        b, hp = core // 4, core % 4
        in_maps.append({
            "x_b": xb[b],
            "wq_s": np.ascontiguousarray(Wq[:, 128 * hp:128 * hp + 128]).astype(BF),
            "wk_s": np.ascontiguousarray(Wk[:, 128 * hp:128 * hp + 128]).astype(BF),
            "wv_s": np.ascontiguousarray(Wv[:, 384 * hp:384 * hp + 384]).astype(BF),
            "wrk_s": np.ascontiguousarray(
                W_rel_k[:, 128 * hp:128 * hp + 128]).astype(BF),
            "posT": posT,
            "bc_s": np.ascontiguousarray(bc[2 * hp:2 * hp + 2].reshape(128)),
            "bp_s": np.ascontiguousarray(bp[2 * hp:2 * hp + 2].reshape(128)),
            "wo_s": np.ascontiguousarray(Wo[384 * hp:384 * hp + 384]).astype(BF),
        })
    return in_maps


def kernel(x, Wq, Wk, Wv, W_rel_k, rel_content_bias, rel_pos_bias, Wo, bo):
    bo = np.asarray(bo, np.float32)
    in_maps = make_in_maps(dict(
        x=x, Wq=Wq, Wk=Wk, Wv=Wv, W_rel_k=W_rel_k,
        rel_content_bias=rel_content_bias, rel_pos_bias=rel_pos_bias, Wo=Wo))
    nc = _get_nc()
    res = run_bass_kernel_spmd(nc, in_maps, list(range(8)))
    out = np.zeros((2, N, DIM), np.float32)
    for core in range(8):
        o = np.asarray(res.results[core]["out_p"])
        if o.dtype == np.uint16:
            o = o.view(BF)
        out[core // 4] += o.astype(np.float32)
    out += bo
    return out
